# revision 15
# baseline (speedup 1.0000x reference)
"""Trainium2 Bass kernel for nn_Lowpass: per-128-block RBJ lowpass biquad.

Algorithm (per 128-sample block, zero initial state):
  y = IIR(FIR(x)) with per-block coefficients from avg-pooled control params.
  FIR: u[n] = x[n] + 2 x[n-1] + x[n-2]    (b0 factored out; b2 == b0, b1 == 2 b0)
  IIR poles are complex (r e^{+-i theta}).  Rotated-frame decomposition turns
  the order-2 recurrence into two real first-order scans that map directly to
  the DVE tensor_tensor_scan instruction:
      v_re[n] = r v_re[n-1] + cos(n theta) u[n]
      v_im[n] = r v_im[n-1] - sin(n theta) u[n]
      y[n]    = Z b0 (cos(n theta + phi) v_re[n] - sin(n theta + phi) v_im[n])
  with 2c = 1 - i pr/pi the pole residue, Z = |2c|, phi = arg(2c).
  (The kernel scans d_im = +sin * u, flipping the recombine sign to +.)

Angle grids n*theta/(2 pi) are built on the (otherwise idle) tensor engine as
outer products theta'^T @ blockdiag(iota), range-reduced with the fp32
magic-number round trick on the scalar engine, so the vector engine only pays
one subtract per grid set.

Sharding: pure data parallel, core c processes batches [4c, 4c+4).
"""

import sys

sys.path.insert(0, "/opt/trn_rl_repo")

import math
from contextlib import ExitStack

import numpy as np

import concourse.bacc as bacc
import concourse.bass as bass
import concourse.mybir as mybir
from concourse.tile import TileContext

F32 = mybir.dt.float32
AX = mybir.AxisListType
ALU = mybir.AluOpType
ACT = mybir.ActivationFunctionType

SR = 44100.0
BLOCK = 128
FC_MIN, FC_MAX = 2000.0, 20000.0
Q_MIN, Q_MAX = 0.1, 10.0
PI = math.pi
MAGIC = 1.5 * 2.0 ** 23     # fp32 round-to-nearest-int bias
INV2PI = 1.0 / (2.0 * PI)
TWOPI = 2.0 * PI


def make_consts(NB, S):
    """Host-precomputed constants: block-diag iota/ones rhs + identity."""
    P = 128
    F = S // P
    HI = F // BLOCK
    rhs = np.zeros((64, F), np.float32)
    for g in range(2):
        for h in range(HI):
            rhs[32 * g + h, h * BLOCK:(h + 1) * BLOCK] = np.arange(
                BLOCK, dtype=np.float32)
            rhs[32 * g + 16 + h, h * BLOCK:(h + 1) * BLOCK] = 1.0
    ident = np.eye(128, dtype=np.float32)
    return {"rhs_c": rhs, "ident": ident}


def build_core_kernel(NB=4, S=262144, n_devices=8, dev_clamp=False,
                      magic_on_act=True):
    """Bass kernel for one core: NB batches of S samples."""
    P = 128
    F = S // P            # free elems per row (per batch)
    HI = F // BLOCK       # blocks per partition row
    nblk = S // BLOCK     # blocks per batch

    nc = bacc.Bacc("TRN2", target_bir_lowering=False, debug=False,
                   num_devices=n_devices)
    x_d = nc.dram_tensor("x", [NB, S], F32, kind="ExternalInput")
    cp_d = nc.dram_tensor("cp", [NB, 2, S], F32, kind="ExternalInput")
    rhs_d = nc.dram_tensor("rhs_c", [64, F], F32, kind="ExternalInput")
    id_d = nc.dram_tensor("ident", [128, 128], F32, kind="ExternalInput")
    y_d = nc.dram_tensor("y", [NB, S], F32, kind="ExternalOutput")
    fc_d = nc.dram_tensor("fc", [NB, nblk], F32, kind="ExternalOutput")
    q_d = nc.dram_tensor("q", [NB, nblk], F32, kind="ExternalOutput")

    NBH = NB * HI  # total coeff lanes per partition (<= 64)
    assert NBH <= 64

    with TileContext(nc) as tc, ExitStack() as ctx:
        cpool = ctx.enter_context(tc.tile_pool(name="const", bufs=1))
        spool = ctx.enter_context(tc.tile_pool(name="small", bufs=1))
        cppool = ctx.enter_context(tc.tile_pool(name="cpstage", bufs=3))
        big = ctx.enter_context(tc.tile_pool(name="big", bufs=2))
        psum = ctx.enter_context(tc.tile_pool(name="psum", bufs=1,
                                              space="PSUM"))

        # ---- constants ----
        rhs_sb = cpool.tile([64, F], F32, tag="rhs_sb")
        nc.sync.dma_start(out=rhs_sb[:], in_=rhs_d[:, :])
        id_sb = cpool.tile([128, 128], F32, tag="id_sb")
        nc.sync.dma_start(out=id_sb[:], in_=id_d[:, :])

        _consts = {}

        def c_ap(val):
            if val not in _consts:
                t = cpool.tile([P, 1], F32, tag=f"c{len(_consts)}",
                               name=f"c{len(_consts)}")
                nc.vector.memset(t[:], val)
                _consts[val] = t
            return _consts[val][:]

        # ---- stage 1: pool control params ----
        sum0 = spool.tile([P, NBH], F32, tag="sum0")
        sum1 = spool.tile([P, NBH], F32, tag="sum1")
        for b in range(NB):
            for prm in range(2):
                cpt = cppool.tile([P, F], F32, tag="cpt")
                nc.sync.dma_start(
                    out=cpt[:], in_=cp_d[b, prm].rearrange("(p f) -> p f", p=P))
                dst = (sum0 if prm == 0 else sum1)[:, b * HI:(b + 1) * HI]
                nc.vector.tensor_reduce(
                    dst, cpt[:].rearrange("p (h t) -> p h t", t=BLOCK),
                    axis=AX.X, op=ALU.add)

        def stile(tag):
            return spool.tile([P, NBH], F32, tag=tag, name=tag)

        # ---- coefficient math (ACT calls grouped by table set) ----
        fc = stile("fc")
        nc.vector.tensor_scalar(fc[:], sum0[:], (FC_MAX - FC_MIN) / BLOCK,
                                FC_MIN, op0=ALU.mult, op1=ALU.add)
        q = stile("q")
        nc.vector.tensor_scalar(q[:], sum1[:], (Q_MAX - Q_MIN) / BLOCK,
                                Q_MIN, op0=ALU.mult, op1=ALU.add)
        nc.sync.dma_start(
            out=fc_d.rearrange("b (p h) -> p b h", p=P),
            in_=fc[:].rearrange("p (b h) -> p b h", b=NB))
        nc.sync.dma_start(
            out=q_d.rearrange("b (p h) -> p b h", p=P),
            in_=q[:].rearrange("p (b h) -> p b h", b=NB))

        w0 = stile("w0")
        nc.vector.tensor_scalar(
            w0[:], sum0[:], (FC_MAX - FC_MIN) / BLOCK * 2.0 * PI / SR,
            FC_MIN * 2.0 * PI / SR, op0=ALU.mult, op1=ALU.add)
        # [trig set]
        sinw = stile("sinw")
        nc.scalar.activation(sinw[:], w0[:], ACT.Sin)
        cosw = stile("cosw")
        nc.scalar.activation(cosw[:], w0[:], ACT.Sin, scale=-1.0,
                             bias=c_ap(PI / 2))

        qr = stile("qr")
        nc.vector.reciprocal(qr[:], q[:])
        alpha = stile("alpha")
        nc.vector.scalar_tensor_tensor(alpha[:], sinw[:], 0.5, qr[:],
                                       op0=ALU.mult, op1=ALU.mult)
        t0 = stile("t0")
        nc.vector.tensor_scalar_add(t0[:], alpha[:], 1.0)
        a0r = stile("a0r")
        nc.vector.reciprocal(a0r[:], t0[:])

        t1 = stile("t1")
        nc.vector.tensor_scalar(t1[:], cosw[:], -0.5, 0.5,
                                op0=ALU.mult, op1=ALU.add)
        b0 = stile("b0")
        nc.vector.tensor_tensor(b0[:], t1[:], a0r[:], op=ALU.mult)
        pr = stile("pr")
        nc.vector.tensor_tensor(pr[:], cosw[:], a0r[:], op=ALU.mult)
        t2 = stile("t2")
        nc.vector.tensor_scalar(t2[:], alpha[:], -1.0, 1.0,
                                op0=ALU.mult, op1=ALU.add)
        a2 = stile("a2")
        nc.vector.tensor_tensor(a2[:], t2[:], a0r[:], op=ALU.mult)
        prsq = stile("prsq")
        nc.vector.tensor_tensor(prsq[:], pr[:], pr[:], op=ALU.mult)
        pi2 = stile("pi2")
        nc.vector.tensor_tensor(pi2[:], a2[:], prsq[:], op=ALU.subtract)
        nc.vector.tensor_scalar_max(pi2[:], pi2[:], 1e-12)
        # [ln/exp set, batched] r = sqrt(a2), pi_ = sqrt(pi2)
        ln_a2 = stile("ln_a2")
        nc.scalar.activation(ln_a2[:], a2[:], ACT.Ln)
        ln_p = stile("ln_p")
        nc.scalar.activation(ln_p[:], pi2[:], ACT.Ln)
        r_t = stile("r_t")
        nc.scalar.activation(r_t[:], ln_a2[:], ACT.Exp, scale=0.5)
        pi_ = stile("pi_")
        nc.scalar.activation(pi_[:], ln_p[:], ACT.Exp, scale=0.5)

        pir = stile("pir")
        nc.vector.reciprocal(pir[:], pi_[:])
        ratio = stile("ratio")
        nc.vector.tensor_tensor(ratio[:], pr[:], pir[:], op=ALU.mult)
        # [trig set] phi = -atan(ratio); cos(phi) = sin(phi + pi/2)
        atn = stile("atn")
        nc.scalar.activation(atn[:], ratio[:], ACT.Arctan, scale=-1.0)
        cphi = stile("cphi")
        nc.scalar.activation(cphi[:], atn[:], ACT.Sin, bias=c_ap(PI / 2))
        theta = stile("theta")
        nc.vector.tensor_scalar_add(theta[:], atn[:], PI / 2)
        # Z = |2c| = 1/cos(phi) ; ZB = Z * b0
        z_t = stile("z_t")
        nc.vector.reciprocal(z_t[:], cphi[:])
        zb = stile("zb")
        nc.vector.tensor_tensor(zb[:], z_t[:], b0[:], op=ALU.mult)

        # theta' = theta/2pi (cols 0..NBH), phi' = phi/2pi (cols 64..64+NBH)
        # packed into one (128,128) tile and PE-transposed for lhsT slicing.
        # batch b lives at col 64*(b//2) + 32*(b%2): theta' at +[0,HI),
        # phi' at +[16,16+HI).  After the PE transpose those become rows;
        # two 64-row lhsT tiles keep matmul base partitions in {0, 32}.
        tpin = cpool.tile([128, 128], F32, tag="tpin")
        nc.vector.memset(tpin[:], 0.0)
        for b in range(NB):
            c0 = 64 * (b // 2) + 32 * (b % 2)
            nc.vector.tensor_scalar_mul(
                tpin[:, c0:c0 + HI],
                theta[:, b * HI:(b + 1) * HI], INV2PI)
            nc.vector.tensor_scalar_mul(
                tpin[:, c0 + 16:c0 + 16 + HI],
                atn[:, b * HI:(b + 1) * HI], INV2PI)
        ps_t = psum.tile([128, 128], F32, tag="pp", name="ps_t")
        nc.tensor.transpose(ps_t[:], tpin[:], id_sb[:])
        lhsT_a = cpool.tile([64, 128], F32, tag="lhsT_a")
        nc.scalar.copy(lhsT_a[:], ps_t[0:64, :])
        lhsT_b = cpool.tile([64, 128], F32, tag="lhsT_b")
        nc.scalar.copy(lhsT_b[:], ps_t[64:128, :])

        # ---- stage 2: per-batch streaming filter ----
        _bufs2 = {"x", "ang"}

        def bt(tag):
            return big.tile([P, F], F32, tag=tag, name=tag,
                            bufs=2 if tag in _bufs2 else 1)

        for b in range(NB):
            sl = slice(b * HI, (b + 1) * HI)

            x_sb = bt("x")
            nc.sync.dma_start(
                out=x_sb[:], in_=x_d[b].rearrange("(p f) -> p f", p=P))
            xv = x_sb[:].rearrange("p (h t) -> p h t", t=BLOCK)

            # FIR u = x + 2 x_{-1} + x_{-2} (per block; fix cols 0,1)
            u1 = bt("u1")
            nc.vector.scalar_tensor_tensor(
                u1[:, 1:], x_sb[:, :F - 1], 2.0, x_sb[:, 1:],
                op0=ALU.mult, op1=ALU.add)
            nc.vector.tensor_copy(u1[:, 0:1], x_sb[:, 0:1])
            u = bt("u")
            nc.vector.tensor_tensor(u[:, 2:], u1[:, 2:], x_sb[:, :F - 2],
                                    op=ALU.add)
            uv = u[:].rearrange("p (h t) -> p h t", t=BLOCK)
            nc.vector.tensor_copy(uv[:, :, 0:1], xv[:, :, 0:1])
            nc.vector.scalar_tensor_tensor(
                uv[:, :, 1:2], xv[:, :, 0:1], 2.0, xv[:, :, 1:2],
                op0=ALU.mult, op1=ALU.add)

            # angle grids (in turns) via PE:
            #   pa[p, h*128+t] = t * theta'[p, b*HI+h]
            #   pp[p, h*128+t] = t * theta'[p, b*HI+h] + phi'[p, b*HI+h]
            lt = lhsT_a if b < 2 else lhsT_b
            p0 = 32 * (b % 2)
            pa = psum.tile([P, F], F32, tag="pa", name="pa")
            pp = psum.tile([P, F], F32, tag="pp", name="pp")
            for c in range(0, F, 512):
                ce = min(c + 512, F)
                nc.tensor.matmul(pa[:, c:ce], lt[p0:p0 + 16, :],
                                 rhs_sb[p0:p0 + 16, c:ce])
                nc.tensor.matmul(pp[:, c:ce], lt[p0:p0 + 32, :],
                                 rhs_sb[p0:p0 + 32, c:ce])

            def grids(src, ctag, stag):
                # K = round(src); f = src - K in [-.5, .5] (turns)
                tk = bt("tk")
                kk = bt("kk")
                if magic_on_act:
                    nc.scalar.activation(tk[:], src[:], ACT.Abs,
                                         bias=c_ap(MAGIC))
                    nc.scalar.activation(kk[:], tk[:], ACT.Abs,
                                         bias=c_ap(-MAGIC))
                else:
                    nc.vector.tensor_scalar_add(tk[:], src[:], MAGIC)
                    nc.vector.tensor_scalar_sub(kk[:], tk[:], MAGIC)
                f = bt("f_" + stag)
                nc.vector.tensor_tensor(f[:], src[:], kk[:], op=ALU.subtract)
                if dev_clamp:
                    nc.vector.tensor_scalar(f[:], f[:], -0.5, 0.5,
                                            op0=ALU.max, op1=ALU.min)
                # sin(2 pi f); cos = sin(pi/2 - 2 pi |f|)
                sgr = bt(stag)
                nc.scalar.activation(sgr[:], f[:], ACT.Sin, scale=TWOPI)
                fa = bt("fa")
                nc.scalar.activation(fa[:], f[:], ACT.Abs)
                cgr = bt(ctag)
                nc.scalar.activation(cgr[:], fa[:], ACT.Sin, scale=-TWOPI,
                                     bias=c_ap(PI / 2))
                return cgr, sgr

            cg, sg = grids(pa, "cg", "sg")      # cos/sin(n theta)
            cpg, spg = grids(pp, "cpg", "spg")  # cos/sin(n theta + phi)

            # scan multiplier grid: r per lane, 0 at block starts
            d0 = bt("d0")
            d0v = d0[:].rearrange("p (h t) -> p h t", t=BLOCK)
            r_b = r_t[:, sl].unsqueeze(2).broadcast_to((P, HI, BLOCK))
            nc.scalar.activation(d0v, r_b, ACT.Copy)
            nc.vector.memset(d0v[:, :, 0:1], 0.0)

            dre = bt("dre")
            nc.vector.tensor_tensor(dre[:], cg[:], u[:], op=ALU.mult)
            dim = bt("dim")
            nc.vector.tensor_tensor(dim[:], sg[:], u[:], op=ALU.mult)

            vre = bt("vre")
            nc.vector.tensor_tensor_scan(vre[:], d0[:], dre[:], 0.0,
                                         op0=ALU.mult, op1=ALU.add)
            vim = bt("vim")
            nc.vector.tensor_tensor_scan(vim[:], d0[:], dim[:], 0.0,
                                         op0=ALU.mult, op1=ALU.add)

            # y = ZB * (cos(psi) v_re + sin(psi) v_im')   [v_im' = -v_im]
            m1 = bt("dre")
            nc.vector.tensor_tensor(m1[:], cpg[:], vre[:], op=ALU.mult)
            m2 = bt("dim")
            nc.vector.tensor_tensor(m2[:], spg[:], vim[:], op=ALU.mult)
            s = bt("u")
            nc.vector.tensor_tensor(s[:], m1[:], m2[:], op=ALU.add)
            y = bt("ang")
            zb_b = zb[:, sl].unsqueeze(2).broadcast_to((P, HI, BLOCK))
            yv = y[:].rearrange("p (h t) -> p h t", t=BLOCK)
            nc.vector.tensor_tensor(yv, s[:].rearrange(
                "p (h t) -> p h t", t=BLOCK), zb_b, op=ALU.mult)

            nc.sync.dma_start(
                out=y_d[b].rearrange("(p f) -> p f", p=P), in_=y[:])

    nc.compile()
    return nc


_NC_CACHE = {}


def _get_nc(NB, S, **kw):
    key = (NB, S, tuple(sorted(kw.items())))
    if key not in _NC_CACHE:
        _NC_CACHE[key] = build_core_kernel(NB, S, **kw)
    return _NC_CACHE[key]


def kernel(x: np.ndarray, control_params: np.ndarray):
    """Full-input entry: x (32,1,262144), control_params (32,2,262144).
    Returns (out, fc, q) matching reference."""
    from concourse.bass_utils import run_bass_kernel_spmd

    B, _, S = x.shape
    n_cores = 8
    nb = B // n_cores
    nblk = S // BLOCK
    nc = _get_nc(nb, S)
    consts = make_consts(nb, S)

    x2 = np.ascontiguousarray(x[:, 0, :], dtype=np.float32)
    cp = np.ascontiguousarray(control_params, dtype=np.float32)
    in_maps = [
        {"x": x2[c * nb:(c + 1) * nb], "cp": cp[c * nb:(c + 1) * nb], **consts}
        for c in range(n_cores)
    ]
    res = run_bass_kernel_spmd(nc, in_maps, list(range(n_cores)))

    out = np.empty((B, 1, S), dtype=np.float32)
    fc = np.empty((B, nblk), dtype=np.float32)
    q = np.empty((B, nblk), dtype=np.float32)
    for c in range(n_cores):
        rd = res.results[c]
        out[c * nb:(c + 1) * nb, 0, :] = rd["y"]
        fc[c * nb:(c + 1) * nb] = rd["fc"]
        q[c * nb:(c + 1) * nb] = rd["q"]
    return out, fc, q
